# revision 3
# baseline (speedup 1.0000x reference)
"""Trainium2 Bass kernel for nn_AttentionOverride (GPT-2 attention block with
attention-weight override), SPMD across 8 NeuronCores.

Sharding: tensor-parallel by head. Core c owns global heads (2c, 2c+1):
  - qkv projection column-split (each core computes q,k,v for its 2 heads)
  - attention (scores, softmax, override merge, AV) fully local per head
  - attention outputs are produced feature-major and exchanged with an
    AllToAll so each core ends up with a row-slice (b,s) of the full
    activation, on which it runs the full-contraction c_proj.

Compute dtype bf16 (f32 PSUM accumulation); scores are built as
[key, query] tiles so exp/causal/override/AV need no transposes. The
softmax denominator is a ones-column matmul packed next to the AV matmul;
normalization is applied on the small [64, 512] AV outputs.
"""

import numpy as np
import ml_dtypes

B, S, D, H = 4, 1024, 1024, 16
HD = D // H  # 64
N_CORES = 8
HPC = H // N_CORES  # heads per core = 2

BF16 = ml_dtypes.bfloat16

_CACHE = {}


def _build_program():
    import concourse.bass as bass
    import concourse.mybir as mybir
    import concourse.tile as tile
    from concourse import bacc

    f32 = mybir.dt.float32
    bf16 = mybir.dt.bfloat16
    AF = mybir.ActivationFunctionType
    OP = mybir.AluOpType

    nc = bacc.Bacc("TRN2", target_bir_lowering=False, debug=False,
                   num_devices=N_CORES)

    # ---- I/O ----
    xbf = nc.declare_dram_parameter("xbf", [B, S, D], bf16, isOutput=False)
    w_qk = nc.declare_dram_parameter("w_qk", [D, 256], bf16, isOutput=False)
    b_qk = nc.declare_dram_parameter("b_qk", [128, 2], f32, isOutput=False)
    w_kv = nc.declare_dram_parameter("w_kv", [D, 256], bf16, isOutput=False)
    b_kv = nc.declare_dram_parameter("b_kv", [1, 256], bf16, isOutput=False)
    w_pj = nc.declare_dram_parameter("w_pj", [D, D], bf16, isOutput=False)
    b_pj = nc.declare_dram_parameter("b_pj", [1, D], bf16, isOutput=False)
    ovrT = nc.declare_dram_parameter("ovrT", [HPC, S, S], bf16, isOutput=False)
    mT = nc.declare_dram_parameter("mT", [HPC, S, S], bf16, isOutput=False)

    present = nc.declare_dram_parameter(
        "present", [2, B, HPC, S, HD], f32, isOutput=True)
    a_out = nc.declare_dram_parameter("a_out", [512, D], f32, isOutput=True)

    with tile.TileContext(nc) as tc:
        _emit(nc, tc, tile, mybir, bass, dict(
            xbf=xbf, w_qk=w_qk, b_qk=b_qk, w_kv=w_kv, b_kv=b_kv,
            w_pj=w_pj, b_pj=b_pj, ovrT=ovrT, mT=mT,
            present=present, a_out=a_out,
        ), f32, bf16, AF, OP)

    nc.compile()
    return nc


def _emit(nc, tc, tile, mybir, bass, io, f32, bf16, AF, OP):
    from contextlib import ExitStack

    xbf, w_qk, b_qk, w_kv, b_kv = (io["xbf"], io["w_qk"], io["b_qk"],
                                   io["w_kv"], io["b_kv"])
    w_pj, b_pj, ovrT, mT = io["w_pj"], io["b_pj"], io["ovrT"], io["mT"]
    present, a_out = io["present"], io["a_out"]

    ctx = ExitStack()
    with ctx:
        const = ctx.enter_context(tc.tile_pool(name="const", bufs=1))
        xtp = ctx.enter_context(tc.tile_pool(name="xtp", bufs=2))
        work = ctx.enter_context(tc.tile_pool(name="work", bufs=3))
        stg = ctx.enter_context(tc.tile_pool(name="stg", bufs=2))
        maskp = ctx.enter_context(tc.tile_pool(name="maskp", bufs=1))
        outp = ctx.enter_context(tc.tile_pool(name="outp", bufs=3))
        psum = ctx.enter_context(tc.tile_pool(name="psum", bufs=2, space="PSUM"))
        dram = ctx.enter_context(tc.tile_pool(name="dram", bufs=1, space="DRAM"))

        # ---- constants ----
        caus = const.tile([128, 4, 512], bf16)  # caus[:, j, :]: keep k<=q-128j
        for j in range(4):
            nc.gpsimd.memset(caus[:, j, :], 1.0)
            nc.gpsimd.affine_select(
                out=caus[:, j, :], in_=caus[:, j, :],
                compare_op=OP.is_ge, fill=0.0,
                base=-128 * j, pattern=[[1, 512]], channel_multiplier=-1)
        onesM = const.tile([128, 64], bf16)   # lhsT for denominator matmul
        nc.gpsimd.memset(onesM[:], 1.0)
        ones1 = const.tile([1, 128], bf16)    # lhsT for rank-1 bias matmuls
        nc.gpsimd.memset(ones1[:], 1.0)

        wqk_sb = const.tile([128, 8, 256], bf16)
        nc.sync.dma_start(wqk_sb[:], w_qk.ap().rearrange("(t p) e -> p t e", p=128))
        wkv_sb = const.tile([128, 8, 256], bf16)
        nc.sync.dma_start(wkv_sb[:], w_kv.ap().rearrange("(t p) e -> p t e", p=128))
        bqk_sb = const.tile([128, 2], f32)
        nc.sync.dma_start(bqk_sb[:], b_qk.ap())
        bkv_sb = const.tile([1, 256], bf16)
        nc.sync.dma_start(bkv_sb[:], b_kv.ap())
        wpj_sb = const.tile([128, 8, 1024], bf16)
        nc.sync.dma_start(wpj_sb[:], w_pj.ap().rearrange("(t p) e -> p t e", p=128))
        bpj_sb = const.tile([1, 1024], bf16)
        nc.sync.dma_start(bpj_sb[:], b_pj.ap())

        # persistent activations: feature-major q/k (h0 on partitions 0-63,
        # h1 on 64-127), token-major v per (batch, key-tile)
        q_fm = const.tile([128, B * S], bf16)
        k_fm = const.tile([128, B * S], bf16)
        v_bf = const.tile([128, B, 8, 128], bf16)

        # ---- phase 1: qkv projections ----
        for sc in range(8):          # s-chunks of 512 rows (b, s-half)
            b, sh = sc // 2, sc % 2
            xt = xtp.tile([128, 8, 512], bf16, tag="xt")
            for dt in range(8):
                nc.sync.dma_start(
                    out=xt[:, dt, :],
                    in_=xbf.ap()[b, sh * 512:(sh + 1) * 512,
                                 dt * 128:(dt + 1) * 128],
                    transpose=True)
            # feature-major q (m=0) and k (m=1) for both heads
            for m in range(2):
                p1 = psum.tile([128, 512], f32, tag="sc")
                for dt in range(8):
                    nc.tensor.matmul(p1[:], wqk_sb[:, dt, m * 128:(m + 1) * 128],
                                     xt[:, dt, :], start=(dt == 0), stop=(dt == 7))
                dst = q_fm if m == 0 else k_fm
                for hl in range(2):
                    nc.scalar.activation(
                        out=dst[hl * 64:(hl + 1) * 64, sc * 512:(sc + 1) * 512],
                        in_=p1[hl * 64:(hl + 1) * 64, :],
                        func=AF.Identity,
                        bias=bqk_sb[hl * 64:(hl + 1) * 64, m:m + 1], scale=1.0)
            # token-major k,v (k only feeds `present`; v feeds AV + present)
            kvf = stg.tile([128, 4, 256], f32, tag="kvf")
            for st in range(4):
                p2 = psum.tile([128, 256], f32, tag="sc")
                for dt in range(8):
                    nc.tensor.matmul(p2[:], xt[:, dt, st * 128:(st + 1) * 128],
                                     wkv_sb[:, dt, :], start=(dt == 0), stop=False)
                nc.tensor.matmul(p2[:], ones1[:, 0:128], bkv_sb[:],
                                 start=False, stop=True)
                nc.scalar.copy(kvf[:, st, :], p2[:])
                nc.vector.tensor_copy(
                    v_bf[:, b, sh * 4 + st, :], p2[:, 128:256])
            for t in range(2):       # 0: k -> present[0], 1: v -> present[1]
                for hl in range(2):
                    nc.sync.dma_start(
                        out=present.ap()[t, b, hl, sh * 512:(sh + 1) * 512, :]
                        .rearrange("(st p) hd -> p st hd", p=128),
                        in_=kvf[:, :, 128 * t + 64 * hl: 128 * t + 64 * hl + 64])

        # ---- A2A bounce buffers ----
        in_a2a = []
        out_a2a = []
        for hl in range(2):
            ia = dram.tile([8, 64, 512], bf16, name=f"in_a2a_{hl}")
            oa = dram.tile([8, 64, 512], bf16, name=f"out_a2a_{hl}")
            in_a2a.append(ia)
            out_a2a.append(oa)

        # ---- phase 2: attention per local head ----
        for hl in range(2):
            m_sb = maskp.tile([128, 8, 1024], bf16, tag="m")
            nc.sync.dma_start(m_sb[:], mT.ap()[hl].rearrange(
                "(t p) q -> p t q", p=128))
            ov_sb = maskp.tile([128, 8, 1024], bf16, tag="ov")
            nc.sync.dma_start(ov_sb[:], ovrT.ap()[hl].rearrange(
                "(t p) q -> p t q", p=128))
            om1 = maskp.tile([128, 8, 1024], bf16, tag="om1")  # 1 - mask
            nc.vector.tensor_scalar(om1[:], m_sb[:], -1.0, 1.0, OP.mult, OP.add)
            omv = maskp.tile([128, 8, 1024], bf16, tag="omv")  # mask * override
            nc.vector.tensor_tensor(omv[:], m_sb[:], ov_sb[:], OP.mult)

            for b in range(B):
                for qc in range(2):
                    nvalid = 4 if qc == 0 else 8
                    Yp = psum.tile([64, 512], f32, tag="Y")
                    Op = psum.tile([64, 512], f32, tag="O")
                    dp = psum.tile([64, 512], f32, tag="dd")
                    for kt in range(8):
                        if kt < nvalid:
                            sc_p = psum.tile([128, 512], f32, tag="sc")
                            nc.tensor.matmul(
                                sc_p[:],
                                k_fm[64 * hl:64 * hl + 64,
                                     b * S + kt * 128: b * S + (kt + 1) * 128],
                                q_fm[64 * hl:64 * hl + 64,
                                     b * S + qc * 512: b * S + (qc + 1) * 512],
                                start=True, stop=True)
                            E = work.tile([128, 512], bf16, tag="E")
                            nc.scalar.activation(E[:], sc_p[:], AF.Exp,
                                                 scale=0.125)
                            j = kt - 4 * qc
                            if 0 <= j <= 3:  # diagonal tile: causal mask
                                nc.vector.tensor_tensor(
                                    E[:], E[:], caus[:, j, :], OP.mult)
                            Ee = work.tile([128, 512], bf16, tag="Ee")
                            nc.vector.tensor_tensor(
                                Ee[:], E[:],
                                om1[:, kt, qc * 512:(qc + 1) * 512], OP.mult)
                            nc.tensor.matmul(
                                Yp[:], v_bf[:, b, kt, 64 * hl:64 * hl + 64],
                                Ee[:], start=(kt == 0), stop=(kt == nvalid - 1))
                            nc.tensor.matmul(
                                dp[:], onesM[:], E[:],
                                start=(kt == 0), stop=(kt == nvalid - 1))
                        nc.tensor.matmul(
                            Op[:], v_bf[:, b, kt, 64 * hl:64 * hl + 64],
                            omv[:, kt, qc * 512:(qc + 1) * 512],
                            start=(kt == 0), stop=(kt == 7))
                    r_sb = work.tile([64, 512], f32, tag="r")
                    nc.vector.reciprocal(r_sb[:], dp[:])
                    a_sb = outp.tile([64, 512], bf16, tag="a")
                    nc.vector.tensor_tensor(a_sb[:], Yp[:], r_sb[:], OP.mult)
                    nc.vector.tensor_tensor(a_sb[:], a_sb[:], Op[:], OP.add)
                    nc.sync.dma_start(out=in_a2a[hl][2 * b + qc], in_=a_sb[:])

            nc.gpsimd.collective_compute(
                "AllToAll", OP.bypass,
                replica_groups=[list(range(N_CORES))],
                ins=[in_a2a[hl].opt()], outs=[out_a2a[hl].opt()])

        # ---- phase 3: c_proj on this core's 512-row slice ----
        for i in range(8):
            ap_t = outp.tile([128, 512], bf16, tag="aproj", bufs=8)
            nc.sync.dma_start(ap_t[0:64, :], out_a2a[0][i])
            nc.sync.dma_start(ap_t[64:128, :], out_a2a[1][i])
            if i == 0:
                ap_tiles = []
            ap_tiles.append(ap_t)
        for mt in range(4):
            for ncol in range(2):
                pp = psum.tile([128, 512], f32, tag="sc")
                for i in range(8):
                    nc.tensor.matmul(
                        pp[:], ap_tiles[i][:, mt * 128:(mt + 1) * 128],
                        wpj_sb[:, i, ncol * 512:(ncol + 1) * 512],
                        start=(i == 0), stop=False)
                nc.tensor.matmul(pp[:], ones1[:, 0:128],
                                 bpj_sb[:, ncol * 512:(ncol + 1) * 512],
                                 start=False, stop=True)
                ao = outp.tile([128, 512], f32, tag="ao")
                nc.scalar.copy(ao[:], pp[:])
                nc.sync.dma_start(
                    out=a_out.ap()[mt * 128:(mt + 1) * 128,
                                   ncol * 512:(ncol + 1) * 512],
                    in_=ao[:])


def _shard_inputs(x, w_attn, b_attn, w_proj, b_proj, attn_override,
                  attn_override_mask):
    """Slice/cast/transpose the full inputs into 8 per-core input maps."""
    xbf = np.ascontiguousarray(x.astype(BF16))
    w_pj = np.ascontiguousarray(w_proj.astype(BF16))
    b_pj = np.ascontiguousarray(b_proj.astype(BF16)).reshape(1, D)
    in_maps = []
    for c in range(N_CORES):
        hs = [2 * c, 2 * c + 1]
        qcols = [w_attn[:, 64 * h:64 * h + 64] for h in hs]
        kcols = [w_attn[:, D + 64 * h:D + 64 * h + 64] for h in hs]
        vcols = [w_attn[:, 2 * D + 64 * h:2 * D + 64 * h + 64] for h in hs]
        w_qk = np.ascontiguousarray(
            np.concatenate(qcols + kcols, axis=1).astype(BF16))
        w_kv = np.ascontiguousarray(
            np.concatenate(kcols + vcols, axis=1).astype(BF16))
        bq = [b_attn[64 * h:64 * h + 64] for h in hs]
        bk = [b_attn[D + 64 * h:D + 64 * h + 64] for h in hs]
        bv = [b_attn[2 * D + 64 * h:2 * D + 64 * h + 64] for h in hs]
        b_qk = np.stack([np.concatenate(bq), np.concatenate(bk)],
                        axis=1).astype(np.float32)
        b_kv = np.concatenate(bk + bv).astype(BF16).reshape(1, 256)
        ovrT_ = np.ascontiguousarray(
            np.stack([attn_override[h].T for h in hs]).astype(BF16))
        mT_ = np.ascontiguousarray(
            np.stack([attn_override_mask[h].T for h in hs])
            .astype(np.float32).astype(BF16))
        in_maps.append(dict(xbf=xbf, w_qk=w_qk, b_qk=b_qk, w_kv=w_kv,
                            b_kv=b_kv, w_pj=w_pj, b_pj=b_pj, ovrT=ovrT_,
                            mT=mT_))
    return in_maps


def get_program():
    if "nc" not in _CACHE:
        _CACHE["nc"] = _build_program()
    return _CACHE["nc"]


def _assemble(results):
    a = np.empty((B, S, D), dtype=np.float32)
    pres = np.empty((2, B, H, S, HD), dtype=np.float32)
    for c in range(N_CORES):
        r = results[c]
        b, sh = c // 2, c % 2
        a[b, sh * 512:(sh + 1) * 512, :] = r["a_out"]
        pres[:, :, 2 * c:2 * c + 2] = r["present"]
    return a, pres


def kernel(**inputs):
    from concourse import bass_utils

    inputs = {k: np.asarray(v) for k, v in inputs.items()}
    nc = get_program()
    in_maps = _shard_inputs(**inputs)
    res = bass_utils.run_bass_kernel_spmd(
        nc, in_maps, core_ids=list(range(N_CORES)))
    return _assemble(res.results)


# revision 7
# speedup vs baseline: 1.0208x; 1.0208x over previous
"""Trainium2 Bass kernel for nn_AttentionOverride (GPT-2 attention block with
attention-weight override), SPMD across 8 NeuronCores.

Sharding: tensor-parallel by head. Core c owns global heads (2c, 2c+1):
  - qkv projection column-split (each core computes q,k,v for its 2 heads)
  - attention fully local per head; the two local heads are processed as a
    pair occupying disjoint halves of the PE array (rows 0-63/64-127 for the
    K=64 score matmuls, columns 0-63/64-127 for the M=64 AV matmuls via
    tile_position) so their matmuls run concurrently
  - attention outputs are produced feature-major and exchanged with a single
    AllToAll so each core ends up with a 512-row (b,s) slice of the full
    activation, on which it runs the full-contraction c_proj.

Compute dtype bf16 (f32 PSUM accumulation). Scores are built as
[key, query] tiles: exp on ScalarE (free 1/8 scale), causal masking via
GPSIMD affine_select, override merge as (1-m)*E and (m*override) @ V with
the softmax denominator from a packed ones-matmul; normalization uses
reciprocal_approx_fast on the [128,512] head-pair AV outputs.
"""

import numpy as np
import ml_dtypes

B, S, D, H = 4, 1024, 1024, 16
HD = D // H  # 64
N_CORES = 8
HPC = H // N_CORES  # heads per core = 2

BF16 = ml_dtypes.bfloat16

_CACHE = {}


def _build_program():
    import concourse.bass as bass
    import concourse.mybir as mybir
    import concourse.tile as tile
    from concourse import bacc

    f32 = mybir.dt.float32
    bf16 = mybir.dt.bfloat16
    AF = mybir.ActivationFunctionType
    OP = mybir.AluOpType

    nc = bacc.Bacc("TRN2", target_bir_lowering=False, debug=False,
                   num_devices=N_CORES)

    # ---- I/O ----
    xbf = nc.declare_dram_parameter("xbf", [B, S, D], bf16, isOutput=False)
    w_qk = nc.declare_dram_parameter("w_qk", [D, 256], bf16, isOutput=False)
    b_qk = nc.declare_dram_parameter("b_qk", [128, 2], f32, isOutput=False)
    w_kv = nc.declare_dram_parameter("w_kv", [D, 256], bf16, isOutput=False)
    b_kv = nc.declare_dram_parameter("b_kv", [1, 256], bf16, isOutput=False)
    w_pj = nc.declare_dram_parameter("w_pj", [D, D], bf16, isOutput=False)
    b_pj = nc.declare_dram_parameter("b_pj", [1, D], bf16, isOutput=False)
    ovrT = nc.declare_dram_parameter("ovrT", [HPC, S, S], bf16, isOutput=False)
    mT = nc.declare_dram_parameter("mT", [HPC, S, S], bf16, isOutput=False)

    present = nc.declare_dram_parameter(
        "present", [2, B, HPC, S, HD], f32, isOutput=True)
    a_out = nc.declare_dram_parameter("a_out", [512, D], f32, isOutput=True)

    with tile.TileContext(nc) as tc:
        _emit(nc, tc, tile, mybir, bass, dict(
            xbf=xbf, w_qk=w_qk, b_qk=b_qk, w_kv=w_kv, b_kv=b_kv,
            w_pj=w_pj, b_pj=b_pj, ovrT=ovrT, mT=mT,
            present=present, a_out=a_out,
        ), f32, bf16, AF, OP)

    nc.compile()
    return nc


def _emit(nc, tc, tile, mybir, bass, io, f32, bf16, AF, OP):
    from contextlib import ExitStack

    xbf, w_qk, b_qk, w_kv, b_kv = (io["xbf"], io["w_qk"], io["b_qk"],
                                   io["w_kv"], io["b_kv"])
    w_pj, b_pj, ovrT, mT = io["w_pj"], io["b_pj"], io["ovrT"], io["mT"]
    present, a_out = io["present"], io["a_out"]

    ctx = ExitStack()
    with ctx:
        const = ctx.enter_context(tc.tile_pool(name="const", bufs=1))
        xtp = ctx.enter_context(tc.tile_pool(name="xtp", bufs=2))
        work = ctx.enter_context(tc.tile_pool(name="work", bufs=3))
        stg = ctx.enter_context(tc.tile_pool(name="stg", bufs=2))
        maskp = ctx.enter_context(tc.tile_pool(name="maskp", bufs=1))
        outp = ctx.enter_context(tc.tile_pool(name="outp", bufs=3))
        psum = ctx.enter_context(tc.tile_pool(name="psum", bufs=1, space="PSUM"))
        dram = ctx.enter_context(tc.tile_pool(name="dram", bufs=1, space="DRAM"))

        onesM = const.tile([128, 64], bf16)   # lhsT for denominator matmul
        nc.gpsimd.memset(onesM[:], 1.0)
        ones1 = const.tile([1, 128], bf16)    # lhsT for rank-1 bias matmuls
        nc.gpsimd.memset(ones1[:], 1.0)

        wqk_sb = const.tile([128, 8, 256], bf16)
        nc.sync.dma_start(wqk_sb[:], w_qk.ap().rearrange("(t p) e -> p t e", p=128))
        wkv_sb = const.tile([128, 8, 256], bf16)
        nc.sync.dma_start(wkv_sb[:], w_kv.ap().rearrange("(t p) e -> p t e", p=128))
        bqk_sb = const.tile([128, 2], f32)
        nc.sync.dma_start(bqk_sb[:], b_qk.ap())
        bkv_sb = const.tile([1, 256], bf16)
        nc.sync.dma_start(bkv_sb[:], b_kv.ap())
        wpj_sb = const.tile([128, 8, 1024], bf16)
        nc.sync.dma_start(wpj_sb[:], w_pj.ap().rearrange("(t p) e -> p t e", p=128))
        bpj_sb = const.tile([1, 1024], bf16)
        nc.sync.dma_start(bpj_sb[:], b_pj.ap())

        # persistent activations: feature-major q/k (h0 on partitions 0-63,
        # h1 on 64-127), token-major v per (batch, key-tile)
        q_fm = const.tile([128, B * S], bf16)
        k_fm = const.tile([128, B * S], bf16)
        v_bf = const.tile([128, B, 8, 128], bf16)

        # ---- phase 1: qkv projections (per batch) ----
        for b in range(B):
            xt = xtp.tile([128, 8, 1024], bf16, tag="xt")
            for dt in range(8):
                nc.sync.dma_start(
                    out=xt[:, dt, :],
                    in_=xbf.ap()[b, :, dt * 128:(dt + 1) * 128],
                    transpose=True)
            # feature-major q (m=0) and k (m=1), both heads stacked
            for sh in range(2):
                for m in range(2):
                    p1 = psum.tile([128, 512], f32, tag="sc", bufs=2)
                    for dt in range(8):
                        nc.tensor.matmul(
                            p1[:], wqk_sb[:, dt, m * 128:(m + 1) * 128],
                            xt[:, dt, sh * 512:(sh + 1) * 512],
                            start=(dt == 0), stop=(dt == 7))
                    dst = q_fm if m == 0 else k_fm
                    nc.scalar.activation(
                        out=dst[:, b * S + sh * 512: b * S + (sh + 1) * 512],
                        in_=p1[:], func=AF.Identity,
                        bias=bqk_sb[:, m:m + 1], scale=1.0)
            # token-major k,v (k only feeds `present`; v feeds AV + present)
            for sh in range(2):
                kvb = stg.tile([128, 4, 256], bf16, tag="kvb")
                for st in range(4):
                    p2 = psum.tile([128, 256], f32, tag="sc", bufs=2)
                    for dt in range(8):
                        nc.tensor.matmul(
                            p2[:], xt[:, dt, sh * 512 + st * 128:
                                      sh * 512 + (st + 1) * 128],
                            wkv_sb[:, dt, :], start=(dt == 0), stop=False)
                    nc.tensor.matmul(p2[:], ones1[:, 0:128], bkv_sb[:],
                                     start=False, stop=True)
                    nc.vector.tensor_copy(kvb[:, st, :], p2[:])
                    nc.vector.tensor_copy(
                        v_bf[:, b, sh * 4 + st, :], p2[:, 128:256])
                for t in range(2):   # 0: k -> present[0], 1: v -> present[1]
                    for hl in range(2):
                        nc.gpsimd.dma_start(
                            out=present.ap()[t, b, hl,
                                             sh * 512:(sh + 1) * 512, :]
                            .rearrange("(st p) hd -> p st hd", p=128),
                            in_=kvb[:, :, 128 * t + 64 * hl:
                                    128 * t + 64 * hl + 64])

        # ---- masks: paired layout [p, qc, kt, 1024] (h0 cols 0-511) ----
        # om1 = 1 - mask, omv = mask * override; raw mask/override are
        # streamed through small chunks to bound SBUF.
        om1 = maskp.tile([128, 2, 8, 1024], bf16, tag="om1")
        omv = maskp.tile([128, 2, 8, 1024], bf16, tag="omv")
        for qc in range(2):
            for kp in range(4):  # kt pairs
                ks = slice(2 * kp, 2 * kp + 2)
                m_ch = maskp.tile([128, 2, 1024], bf16, tag="mch", bufs=2)
                ov_ch = maskp.tile([128, 2, 1024], bf16, tag="ovch", bufs=2)
                for hl in range(2):
                    nc.sync.dma_start(
                        m_ch[:, :, hl * 512:(hl + 1) * 512],
                        mT.ap()[hl].rearrange(
                            "(kt p) (qc q) -> p qc kt q",
                            p=128, q=512)[:, qc, ks, :])
                    nc.sync.dma_start(
                        ov_ch[:, :, hl * 512:(hl + 1) * 512],
                        ovrT.ap()[hl].rearrange(
                            "(kt p) (qc q) -> p qc kt q",
                            p=128, q=512)[:, qc, ks, :])
                nc.vector.tensor_scalar(om1[:, qc, ks, :], m_ch[:],
                                        -1.0, 1.0, OP.mult, OP.add)
                nc.vector.tensor_tensor(omv[:, qc, ks, :], m_ch[:],
                                        ov_ch[:], OP.mult)

        # ---- A2A bounce ----
        in_a2a = dram.tile([8, 128, 512], bf16, name="in_a2a")
        out_a2a = dram.tile([8, 128, 512], bf16, name="out_a2a")

        # ---- phase 2: attention, both heads paired ----
        for b in range(B):
            for qc in range(2):
                nvalid = 4 if qc == 0 else 8
                Yp = psum.tile([128, 512], f32, tag="Y", bufs=2)
                Op = psum.tile([128, 512], f32, tag="O")
                dp = psum.tile([128, 512], f32, tag="dd")
                for kt in range(8):
                    if kt < nvalid:
                        scp = psum.tile([128, 1024], f32, tag="sc", bufs=2)
                        for hl in range(2):
                            nc.tensor.matmul(
                                scp[:, hl * 512:(hl + 1) * 512],
                                k_fm[64 * hl:64 * hl + 64,
                                     b * S + kt * 128: b * S + (kt + 1) * 128],
                                q_fm[64 * hl:64 * hl + 64,
                                     b * S + qc * 512: b * S + (qc + 1) * 512],
                                start=True, stop=True)
                        E = work.tile([128, 1024], bf16, tag="E")
                        nc.scalar.activation(E[:], scp[:], AF.Exp, scale=0.125)
                        j = kt - 4 * qc
                        if 0 <= j <= 3:  # diagonal tile: zero causally-invalid
                            nc.gpsimd.affine_select(
                                out=E[:].rearrange("p (h q) -> p h q", h=2),
                                in_=E[:].rearrange("p (h q) -> p h q", h=2),
                                compare_op=OP.is_ge,
                                fill=0.0, base=-128 * j,
                                pattern=[[0, 2], [1, 512]],
                                channel_multiplier=-1)
                        Ee = work.tile([128, 1024], bf16, tag="Ee")
                        nc.vector.tensor_tensor(
                            Ee[:], E[:], om1[:, qc, kt, :], OP.mult)
                        for hl in range(2):
                            nc.tensor.matmul(
                                Yp[64 * hl:64 * hl + 64, :],
                                v_bf[:, b, kt, 64 * hl:64 * hl + 64],
                                Ee[:, hl * 512:(hl + 1) * 512],
                                start=(kt == 0), stop=(kt == nvalid - 1),
                                tile_position=(0, 64 * hl),
                                skip_group_check=True)
                            nc.tensor.matmul(
                                dp[64 * hl:64 * hl + 64, :], onesM[:],
                                E[:, hl * 512:(hl + 1) * 512],
                                start=(kt == 0), stop=(kt == nvalid - 1),
                                tile_position=(0, 64 * hl),
                                skip_group_check=True)
                    for hl in range(2):
                        nc.tensor.matmul(
                            Op[64 * hl:64 * hl + 64, :],
                            v_bf[:, b, kt, 64 * hl:64 * hl + 64],
                            omv[:, qc, kt, hl * 512:(hl + 1) * 512],
                            start=(kt == 0), stop=(kt == 7),
                            tile_position=(0, 64 * hl),
                            skip_group_check=True)
                r_sb = work.tile([128, 512], f32, tag="r")
                nc.vector.reciprocal_approx_fast(out=r_sb[:], in_=dp[:])
                a_pair = outp.tile([128, 512], bf16, tag="a")
                nc.vector.tensor_tensor(a_pair[:], Yp[:], r_sb[:], OP.mult)
                nc.vector.tensor_tensor(a_pair[:], a_pair[:], Op[:], OP.add)
                nc.sync.dma_start(out=in_a2a[2 * b + qc], in_=a_pair[:])

        nc.gpsimd.collective_compute(
            "AllToAll", OP.bypass,
            replica_groups=[list(range(N_CORES))],
            ins=[in_a2a.opt()], outs=[out_a2a.opt()])

        # ---- phase 3: c_proj on this core's 512-row slice ----
        ap_tiles = []
        for i in range(8):
            ap_t = outp.tile([128, 512], bf16, tag="aproj", bufs=8)
            nc.sync.dma_start(ap_t[:], out_a2a[i])
            ap_tiles.append(ap_t)
        for mt in range(4):
            for ncol in range(2):
                pp = psum.tile([128, 512], f32, tag="sc", bufs=2)
                for i in range(8):
                    nc.tensor.matmul(
                        pp[:], ap_tiles[i][:, mt * 128:(mt + 1) * 128],
                        wpj_sb[:, i, ncol * 512:(ncol + 1) * 512],
                        start=(i == 0), stop=False)
                nc.tensor.matmul(pp[:], ones1[:, 0:128],
                                 bpj_sb[:, ncol * 512:(ncol + 1) * 512],
                                 start=False, stop=True)
                ao = outp.tile([128, 512], f32, tag="ao")
                nc.scalar.copy(ao[:], pp[:])
                nc.sync.dma_start(
                    out=a_out.ap()[mt * 128:(mt + 1) * 128,
                                   ncol * 512:(ncol + 1) * 512],
                    in_=ao[:])


def _shard_inputs(x, w_attn, b_attn, w_proj, b_proj, attn_override,
                  attn_override_mask):
    """Slice/cast/transpose the full inputs into 8 per-core input maps."""
    xbf = np.ascontiguousarray(x.astype(BF16))
    w_pj = np.ascontiguousarray(w_proj.astype(BF16))
    b_pj = np.ascontiguousarray(b_proj.astype(BF16)).reshape(1, D)
    in_maps = []
    for c in range(N_CORES):
        hs = [2 * c, 2 * c + 1]
        qcols = [w_attn[:, 64 * h:64 * h + 64] for h in hs]
        kcols = [w_attn[:, D + 64 * h:D + 64 * h + 64] for h in hs]
        vcols = [w_attn[:, 2 * D + 64 * h:2 * D + 64 * h + 64] for h in hs]
        w_qk = np.ascontiguousarray(
            np.concatenate(qcols + kcols, axis=1).astype(BF16))
        w_kv = np.ascontiguousarray(
            np.concatenate(kcols + vcols, axis=1).astype(BF16))
        bq = [b_attn[64 * h:64 * h + 64] for h in hs]
        bk = [b_attn[D + 64 * h:D + 64 * h + 64] for h in hs]
        bv = [b_attn[2 * D + 64 * h:2 * D + 64 * h + 64] for h in hs]
        b_qk = np.stack([np.concatenate(bq), np.concatenate(bk)],
                        axis=1).astype(np.float32)
        b_kv = np.concatenate(bk + bv).astype(BF16).reshape(1, 256)
        ovrT_ = np.ascontiguousarray(
            np.stack([attn_override[h].T for h in hs]).astype(BF16))
        mT_ = np.ascontiguousarray(
            np.stack([attn_override_mask[h].T for h in hs])
            .astype(np.float32).astype(BF16))
        in_maps.append(dict(xbf=xbf, w_qk=w_qk, b_qk=b_qk, w_kv=w_kv,
                            b_kv=b_kv, w_pj=w_pj, b_pj=b_pj, ovrT=ovrT_,
                            mT=mT_))
    return in_maps


def get_program():
    if "nc" not in _CACHE:
        _CACHE["nc"] = _build_program()
    return _CACHE["nc"]


def _assemble(results):
    a = np.empty((B, S, D), dtype=np.float32)
    pres = np.empty((2, B, H, S, HD), dtype=np.float32)
    for c in range(N_CORES):
        r = results[c]
        b, sh = c // 2, c % 2
        a[b, sh * 512:(sh + 1) * 512, :] = r["a_out"]
        pres[:, :, 2 * c:2 * c + 2] = r["present"]
    return a, pres


def kernel(**inputs):
    from concourse import bass_utils

    inputs = {k: np.asarray(v) for k, v in inputs.items()}
    nc = get_program()
    in_maps = _shard_inputs(**inputs)
    res = bass_utils.run_bass_kernel_spmd(
        nc, in_maps, core_ids=list(range(N_CORES)))
    return _assemble(res.results)


# revision 9
# speedup vs baseline: 1.0855x; 1.0634x over previous
"""Trainium2 Bass kernel for nn_AttentionOverride (GPT-2 attention block with
attention-weight override), SPMD across 8 NeuronCores.

Sharding: tensor-parallel by head. Core c owns global heads (2c, 2c+1):
  - qkv projection column-split (each core computes q,k,v for its 2 heads)
  - attention fully local per head; the two local heads are processed as a
    pair occupying disjoint halves of the PE array (rows 0-63/64-127 for the
    K=64 score matmuls, columns 0-63/64-127 for the M=64 AV matmuls via
    tile_position) so their matmuls run concurrently
  - attention outputs are produced feature-major and exchanged with a single
    AllToAll so each core ends up with a 512-row (b,s) slice of the full
    activation, on which it runs the full-contraction c_proj.

Compute dtype bf16 (f32 PSUM accumulation). Scores are built as
[key, query] tiles: exp on ScalarE (free 1/8 scale), causal masking via
GPSIMD affine_select, override merge as (1-m)*E and (m*override) @ V with
the softmax denominator from a packed ones-matmul; normalization uses
reciprocal_approx_fast on the [128,512] head-pair AV outputs.
"""

import numpy as np
import ml_dtypes

B, S, D, H = 4, 1024, 1024, 16
HD = D // H  # 64
N_CORES = 8
HPC = H // N_CORES  # heads per core = 2

BF16 = ml_dtypes.bfloat16

_CACHE = {}


def _build_program():
    import concourse.bass as bass
    import concourse.mybir as mybir
    import concourse.tile as tile
    from concourse import bacc

    f32 = mybir.dt.float32
    bf16 = mybir.dt.bfloat16
    AF = mybir.ActivationFunctionType
    OP = mybir.AluOpType

    nc = bacc.Bacc("TRN2", target_bir_lowering=False, debug=False,
                   num_devices=N_CORES)

    # ---- I/O ----
    xbf = nc.declare_dram_parameter("xbf", [B, S, D], bf16, isOutput=False)
    w_qk = nc.declare_dram_parameter("w_qk", [D, 256], bf16, isOutput=False)
    b_qk = nc.declare_dram_parameter("b_qk", [128, 2], f32, isOutput=False)
    w_kv = nc.declare_dram_parameter("w_kv", [D, 256], bf16, isOutput=False)
    b_kv = nc.declare_dram_parameter("b_kv", [1, 256], bf16, isOutput=False)
    w_pj = nc.declare_dram_parameter("w_pj", [D, D], bf16, isOutput=False)
    b_pj = nc.declare_dram_parameter("b_pj", [1, D], bf16, isOutput=False)
    ovrT = nc.declare_dram_parameter("ovrT", [HPC, S, S], bf16, isOutput=False)
    mT = nc.declare_dram_parameter("mT", [HPC, S, S], bf16, isOutput=False)

    present = nc.declare_dram_parameter(
        "present", [2, B, HPC, S, HD], f32, isOutput=True)
    a_out = nc.declare_dram_parameter("a_out", [512, D], f32, isOutput=True)

    with tile.TileContext(nc) as tc:
        _emit(nc, tc, tile, mybir, bass, dict(
            xbf=xbf, w_qk=w_qk, b_qk=b_qk, w_kv=w_kv, b_kv=b_kv,
            w_pj=w_pj, b_pj=b_pj, ovrT=ovrT, mT=mT,
            present=present, a_out=a_out,
        ), f32, bf16, AF, OP)

    nc.compile()
    return nc


def _emit(nc, tc, tile, mybir, bass, io, f32, bf16, AF, OP):
    from contextlib import ExitStack

    xbf, w_qk, b_qk, w_kv, b_kv = (io["xbf"], io["w_qk"], io["b_qk"],
                                   io["w_kv"], io["b_kv"])
    w_pj, b_pj, ovrT, mT = io["w_pj"], io["b_pj"], io["ovrT"], io["mT"]
    present, a_out = io["present"], io["a_out"]

    ctx = ExitStack()
    with ctx:
        const = ctx.enter_context(tc.tile_pool(name="const", bufs=1))
        xtp = ctx.enter_context(tc.tile_pool(name="xtp", bufs=3))
        work = ctx.enter_context(tc.tile_pool(name="work", bufs=3))
        stg = ctx.enter_context(tc.tile_pool(name="stg", bufs=2))
        maskp = ctx.enter_context(tc.tile_pool(name="maskp", bufs=1))
        outp = ctx.enter_context(tc.tile_pool(name="outp", bufs=3))
        psum = ctx.enter_context(tc.tile_pool(name="psum", bufs=1, space="PSUM"))
        dram = ctx.enter_context(tc.tile_pool(name="dram", bufs=1, space="DRAM"))

        onesM = const.tile([128, 64], bf16)   # lhsT for denominator matmul
        nc.gpsimd.memset(onesM[:], 1.0)
        ones1 = const.tile([1, 128], bf16)    # lhsT for rank-1 bias matmuls
        nc.gpsimd.memset(ones1[:], 1.0)
        # causal keep-masks (1 valid / 0 invalid), offset j: keep k <= q - 128j
        caus = const.tile([128, 4, 512], bf16)
        for j in range(4):
            nc.gpsimd.memset(caus[:, j, :], 1.0)
            nc.gpsimd.affine_select(
                out=caus[:, j, :], in_=caus[:, j, :],
                compare_op=OP.is_ge, fill=0.0,
                base=-128 * j, pattern=[[1, 512]], channel_multiplier=-1)
        scratch = const.tile([1, 128], f32)
        nc.scalar.activation(scratch[:], ones1[:], AF.Exp)  # prewarm exp table

        wqk_sb = const.tile([128, 8, 256], bf16)
        nc.sync.dma_start(wqk_sb[:], w_qk.ap().rearrange("(t p) e -> p t e", p=128))
        wkv_sb = const.tile([128, 8, 256], bf16)
        nc.sync.dma_start(wkv_sb[:], w_kv.ap().rearrange("(t p) e -> p t e", p=128))
        bqk_sb = const.tile([128, 2], f32)
        nc.sync.dma_start(bqk_sb[:], b_qk.ap())
        bkv_sb = const.tile([1, 256], bf16)
        nc.sync.dma_start(bkv_sb[:], b_kv.ap())

        # persistent activations: feature-major q/k (h0 on partitions 0-63,
        # h1 on 64-127), token-major v per (batch, key-tile)
        q_fm = const.tile([128, B * S], bf16)
        k_fm = const.tile([128, B * S], bf16)
        v_bf = const.tile([128, B, 8, 128], bf16)

        # masks, paired layout [p, qc, kt, 1024] (h0 cols 0-511, h1 512-1023).
        # om1 = (1 - mask) * causal   (causal pre-folded on diagonal tiles)
        # omv = mask * override
        om1 = maskp.tile([128, 2, 8, 1024], bf16, tag="om1")
        omv = maskp.tile([128, 2, 8, 1024], bf16, tag="omv")

        def emit_mask_chunk(qc, kp):
            ks = slice(2 * kp, 2 * kp + 2)
            m_ch = maskp.tile([128, 2, 1024], bf16, tag="mch", bufs=2,
                              name=f"mch_{qc}_{kp}")
            ov_ch = maskp.tile([128, 2, 1024], bf16, tag="ovch", bufs=2,
                               name=f"ovch_{qc}_{kp}")
            for hl in range(2):
                nc.sync.dma_start(
                    m_ch[:, :, hl * 512:(hl + 1) * 512],
                    mT.ap()[hl].rearrange(
                        "(kt p) (qc q) -> p qc kt q",
                        p=128, q=512)[:, qc, ks, :])
                nc.sync.dma_start(
                    ov_ch[:, :, hl * 512:(hl + 1) * 512],
                    ovrT.ap()[hl].rearrange(
                        "(kt p) (qc q) -> p qc kt q",
                        p=128, q=512)[:, qc, ks, :])
            nc.vector.tensor_scalar(om1[:, qc, ks, :], m_ch[:],
                                    -1.0, 1.0, OP.mult, OP.add)
            nc.vector.tensor_tensor(omv[:, qc, ks, :], m_ch[:],
                                    ov_ch[:], OP.mult)
            for kt in range(2 * kp, 2 * kp + 2):
                j = kt - 4 * qc
                if 0 <= j <= 3:  # diagonal tile: fold causal into om1
                    for hl in range(2):
                        nc.vector.tensor_tensor(
                            om1[:, qc, kt, hl * 512:(hl + 1) * 512],
                            om1[:, qc, kt, hl * 512:(hl + 1) * 512],
                            caus[:, j, :], OP.mult)

        mask_chunks = [(qc, kp) for qc in range(2) for kp in range(4)]

        # ---- phase 1: qkv projections, mask loads interleaved ----
        for b in range(B):
            for sh in range(2):
                xt = xtp.tile([128, 8, 512], bf16, tag="xt")
                for dt in range(8):
                    nc.sync.dma_start(
                        out=xt[:, dt, :],
                        in_=xbf.ap()[b, sh * 512:(sh + 1) * 512,
                                     dt * 128:(dt + 1) * 128],
                        transpose=True)
                # feature-major q (m=0) and k (m=1), both heads stacked
                for m in range(2):
                    p1 = psum.tile([128, 512], f32, tag="sc", bufs=2)
                    for dt in range(8):
                        nc.tensor.matmul(
                            p1[:], wqk_sb[:, dt, m * 128:(m + 1) * 128],
                            xt[:, dt, :], start=(dt == 0), stop=(dt == 7))
                    dst = q_fm if m == 0 else k_fm
                    nc.scalar.activation(
                        out=dst[:, b * S + sh * 512: b * S + (sh + 1) * 512],
                        in_=p1[:], func=AF.Identity,
                        bias=bqk_sb[:, m:m + 1], scale=1.0)
                # token-major k,v
                kvb = stg.tile([128, 4, 256], f32, tag="kvb")
                for st in range(4):
                    p2 = psum.tile([128, 256], f32, tag="sc", bufs=2)
                    for dt in range(8):
                        nc.tensor.matmul(
                            p2[:], xt[:, dt, st * 128:(st + 1) * 128],
                            wkv_sb[:, dt, :], start=(dt == 0), stop=False)
                    nc.tensor.matmul(p2[:], ones1[:, 0:128], bkv_sb[:],
                                     start=False, stop=True)
                    nc.scalar.copy(kvb[:, st, :], p2[:])
                    nc.vector.tensor_copy(
                        v_bf[:, b, sh * 4 + st, :], p2[:, 128:256])
                for t in range(2):   # 0: k -> present[0], 1: v -> present[1]
                    for hl in range(2):
                        nc.sync.dma_start(
                            out=present.ap()[t, b, hl,
                                             sh * 512:(sh + 1) * 512, :]
                            .rearrange("(st p) hd -> p st hd", p=128),
                            in_=kvb[:, :, 128 * t + 64 * hl:
                                    128 * t + 64 * hl + 64])
                if mask_chunks:
                    emit_mask_chunk(*mask_chunks.pop(0))

        while mask_chunks:
            emit_mask_chunk(*mask_chunks.pop(0))

        # proj weights loaded after the front-critical DMAs
        wpj_sb = const.tile([128, 8, 1024], bf16)
        nc.sync.dma_start(wpj_sb[:], w_pj.ap().rearrange("(t p) e -> p t e", p=128))
        bpj_sb = const.tile([1, 1024], bf16)
        nc.sync.dma_start(bpj_sb[:], b_pj.ap())

        # ---- A2A bounce ----
        in_a2a = dram.tile([8, 128, 512], bf16, name="in_a2a")
        out_a2a = dram.tile([8, 128, 512], bf16, name="out_a2a")

        # ---- phase 2: attention, both heads paired ----
        for b in range(B):
            for qc in range(2):
                nvalid = 4 if qc == 0 else 8
                Yp = psum.tile([128, 512], f32, tag="Y", bufs=2)
                Op = psum.tile([128, 512], f32, tag="O")
                dp = psum.tile([128, 512], f32, tag="dd")
                for kt in range(8):
                    if kt < nvalid:
                        scp = psum.tile([128, 1024], f32, tag="sc", bufs=2)
                        for hl in range(2):
                            nc.tensor.matmul(
                                scp[:, hl * 512:(hl + 1) * 512],
                                k_fm[64 * hl:64 * hl + 64,
                                     b * S + kt * 128: b * S + (kt + 1) * 128],
                                q_fm[64 * hl:64 * hl + 64,
                                     b * S + qc * 512: b * S + (qc + 1) * 512],
                                start=True, stop=True)
                        E = work.tile([128, 1024], bf16, tag="E")
                        nc.scalar.activation(E[:], scp[:], AF.Exp, scale=0.125)
                        j = kt - 4 * qc
                        if 0 <= j <= 3:  # diagonal: causally-masked copy for d
                            Ec = work.tile([128, 1024], bf16, tag="Ec", bufs=2)
                            nc.gpsimd.affine_select(
                                out=Ec[:].rearrange("p (h q) -> p h q", h=2),
                                in_=E[:].rearrange("p (h q) -> p h q", h=2),
                                compare_op=OP.is_ge,
                                fill=0.0, base=-128 * j,
                                pattern=[[0, 2], [1, 512]],
                                channel_multiplier=-1)
                            drhs = Ec
                        else:
                            drhs = E
                        Ee = work.tile([128, 1024], bf16, tag="Ee")
                        nc.vector.tensor_tensor(
                            Ee[:], E[:], om1[:, qc, kt, :], OP.mult)
                        for hl in range(2):
                            nc.tensor.matmul(
                                Yp[64 * hl:64 * hl + 64, :],
                                v_bf[:, b, kt, 64 * hl:64 * hl + 64],
                                Ee[:, hl * 512:(hl + 1) * 512],
                                start=(kt == 0), stop=(kt == nvalid - 1),
                                tile_position=(0, 64 * hl),
                                skip_group_check=True)
                            nc.tensor.matmul(
                                dp[64 * hl:64 * hl + 64, :], onesM[:],
                                drhs[:, hl * 512:(hl + 1) * 512],
                                start=(kt == 0), stop=(kt == nvalid - 1),
                                tile_position=(0, 64 * hl),
                                skip_group_check=True)
                    for hl in range(2):
                        nc.tensor.matmul(
                            Op[64 * hl:64 * hl + 64, :],
                            v_bf[:, b, kt, 64 * hl:64 * hl + 64],
                            omv[:, qc, kt, hl * 512:(hl + 1) * 512],
                            start=(kt == 0), stop=(kt == 7),
                            tile_position=(0, 64 * hl),
                            skip_group_check=True)
                r_sb = work.tile([128, 512], f32, tag="r", bufs=2)
                nc.vector.reciprocal_approx_fast(out=r_sb[:], in_=dp[:])
                a_pair = outp.tile([128, 512], bf16, tag="a")
                nc.vector.tensor_tensor(a_pair[:], Yp[:], r_sb[:], OP.mult)
                nc.vector.tensor_tensor(a_pair[:], a_pair[:], Op[:], OP.add)
                nc.sync.dma_start(out=in_a2a[2 * b + qc], in_=a_pair[:])

        nc.gpsimd.collective_compute(
            "AllToAll", OP.bypass,
            replica_groups=[list(range(N_CORES))],
            ins=[in_a2a.opt()], outs=[out_a2a.opt()])

        # ---- phase 3: c_proj on this core's 512-row slice ----
        ap_tiles = []
        for i in range(8):
            ap_t = outp.tile([128, 512], bf16, tag="aproj", bufs=8)
            nc.sync.dma_start(ap_t[:], out_a2a[i])
            ap_tiles.append(ap_t)
        for mt in range(4):
            for ncol in range(2):
                pp = psum.tile([128, 512], f32, tag="sc", bufs=2)
                for i in range(8):
                    nc.tensor.matmul(
                        pp[:], ap_tiles[i][:, mt * 128:(mt + 1) * 128],
                        wpj_sb[:, i, ncol * 512:(ncol + 1) * 512],
                        start=(i == 0), stop=False)
                nc.tensor.matmul(pp[:], ones1[:, 0:128],
                                 bpj_sb[:, ncol * 512:(ncol + 1) * 512],
                                 start=False, stop=True)
                ao = outp.tile([128, 512], f32, tag="ao", bufs=2)
                nc.scalar.copy(ao[:], pp[:])
                nc.sync.dma_start(
                    out=a_out.ap()[mt * 128:(mt + 1) * 128,
                                   ncol * 512:(ncol + 1) * 512],
                    in_=ao[:])


def _shard_inputs(x, w_attn, b_attn, w_proj, b_proj, attn_override,
                  attn_override_mask):
    """Slice/cast/transpose the full inputs into 8 per-core input maps."""
    xbf = np.ascontiguousarray(x.astype(BF16))
    w_pj = np.ascontiguousarray(w_proj.astype(BF16))
    b_pj = np.ascontiguousarray(b_proj.astype(BF16)).reshape(1, D)
    in_maps = []
    for c in range(N_CORES):
        hs = [2 * c, 2 * c + 1]
        qcols = [w_attn[:, 64 * h:64 * h + 64] for h in hs]
        kcols = [w_attn[:, D + 64 * h:D + 64 * h + 64] for h in hs]
        vcols = [w_attn[:, 2 * D + 64 * h:2 * D + 64 * h + 64] for h in hs]
        w_qk = np.ascontiguousarray(
            np.concatenate(qcols + kcols, axis=1).astype(BF16))
        w_kv = np.ascontiguousarray(
            np.concatenate(kcols + vcols, axis=1).astype(BF16))
        bq = [b_attn[64 * h:64 * h + 64] for h in hs]
        bk = [b_attn[D + 64 * h:D + 64 * h + 64] for h in hs]
        bv = [b_attn[2 * D + 64 * h:2 * D + 64 * h + 64] for h in hs]
        b_qk = np.stack([np.concatenate(bq), np.concatenate(bk)],
                        axis=1).astype(np.float32)
        b_kv = np.concatenate(bk + bv).astype(BF16).reshape(1, 256)
        ovrT_ = np.ascontiguousarray(
            np.stack([attn_override[h].T for h in hs]).astype(BF16))
        mT_ = np.ascontiguousarray(
            np.stack([attn_override_mask[h].T for h in hs])
            .astype(np.float32).astype(BF16))
        in_maps.append(dict(xbf=xbf, w_qk=w_qk, b_qk=b_qk, w_kv=w_kv,
                            b_kv=b_kv, w_pj=w_pj, b_pj=b_pj, ovrT=ovrT_,
                            mT=mT_))
    return in_maps


def get_program():
    if "nc" not in _CACHE:
        _CACHE["nc"] = _build_program()
    return _CACHE["nc"]


def _assemble(results):
    a = np.empty((B, S, D), dtype=np.float32)
    pres = np.empty((2, B, H, S, HD), dtype=np.float32)
    for c in range(N_CORES):
        r = results[c]
        b, sh = c // 2, c % 2
        a[b, sh * 512:(sh + 1) * 512, :] = r["a_out"]
        pres[:, :, 2 * c:2 * c + 2] = r["present"]
    return a, pres


def kernel(**inputs):
    from concourse import bass_utils

    inputs = {k: np.asarray(v) for k, v in inputs.items()}
    nc = get_program()
    in_maps = _shard_inputs(**inputs)
    res = bass_utils.run_bass_kernel_spmd(
        nc, in_maps, core_ids=list(range(N_CORES)))
    return _assemble(res.results)
